# revision 18
# baseline (speedup 1.0000x reference)
"""Position-attention kernel for Trainium2 (8 NeuronCores, SPMD).

Reference computation (per batch b):
    q = Wq @ x + bq        [32, 4096]
    k = Wk @ x + bk        [32, 4096]
    v = Wv @ x + bv        [256, 4096]
    attn = softmax_j(q_i . k_j)           [4096, 4096]
    out[c, i] = sum_j v[c, j] attn[i, j]
    y = gamma * out + x

Sharding: B=4 batches x 2 query-halves -> 8 cores. Each core computes the
full softmax rows for its 2048 queries against all 4096 keys of its batch.
Host rotates x columns per core so the core's query half is always columns
0:2048 (softmax and the PV contraction are invariant to key/value column
order, as long as K and V use the same order).

Device-side structure (per core):
  - projections in bf16 (x pre-cast on host; weights pre-packed on host).
  - scores computed transposed (sT[j, i]) in PSUM, 4 key-blocks at a time
    packed into PE row-groups 0/32/64/96 via tile_position (the K=32
    contractions run concurrently in the array); kf is stored packed
    ([d + 32*r] rows), q replicated into all four row groups. Each quad's
    scores land in two 2-bank PSUM tiles (rows 0/32 and rows 64/96).
  - score->e conversion produces SHIFTED weights exp(s - 16) in fp8e5m2,
    SPLIT between the ACT engine (true exp with bias, PSUM -> fp8) and the
    DVE (log-domain affine bit trick: u8 = s*4*log2e + const = e5m2 bits of
    ~exp(s-16), saturating at 0 for underflow). The global 2^-16-ish scale
    cancels in the softmax ratio. Neither engine paces the PE.
  - PV in fp8 with DoubleRow (2 fp8 weights/PE cell, K=256 per matmul):
    out[c, i] = sum_j vT[j, c] e[j, i] with vT (fp8e4m3, [Ki=128,Ko=2,c]
    interleave) stationary and e (fp8e5m2, [Ki,Ko,i]) moving; 16 j-double-
    blocks x 2 channel blocks per supertile. A third accumulation chain
    with an all-ones stationary gives den[i] = sum_j e[j, i] replicated on
    all 128 partitions. Chains are ordered c0-chain, c1-chain, den-chain
    so PSUM out banks recycle without stalling.
  - epilogue: y[c, i] = (out[c, i] * (1/den)[c, i]) * gamma + xpbT[c, i],
    where xpbT = x + gamma*bv (the bv term works because sum_j attn = 1).
    Output written [C, NH] in bf16; host concatenates and upcasts.
  - input DMA issue is split across the sync + scalar HWDGE sequencers
    (~600ns per dma_start, 16 queues each); dummy matmuls + a dummy exp
    warm the PE HAM clock gate and the ACT function table during the DMA
    window.
"""

import os
import numpy as np

P = 128
B = 4
C = 256
CQ = 32
H = W = 64
N = H * W            # 4096 keys per batch
NH = N // 2          # 2048 queries per core
NCB = C // P         # 2 channel blocks
ST = 512             # query supertile
NST = NH // ST       # 4
JB = N // P          # 32 key blocks
JD = JB // 2         # 16 key double-blocks (DoubleRow K=256)
NQ = JB // 4         # 8 score quads per supertile

EXP_K = 16.0                      # softmax shift: weights are exp(s - 16)
EXP_A8 = 5.770780163555856        # 4 * log2(e)
EXP_B8 = 60.0 - EXP_K * EXP_A8    # e5m2 bits offset (60 = 15*4 bias)


RECIP_MAGIC = float(0x7EF127EA)   # bit-trick reciprocal seed constant


# exp offload: half-tiles where _use_dve is True are computed on the DVE.
def _use_dve(st_i, q, half):
    if half == 0:
        return False
    if st_i == 0:
        return q in (2, 5, 7)
    return q not in (1, 3)


_PROG = None         # cached build
LAST_RESULT = None   # BassKernelResults of the last run (for test harness)


def _build_program():
    import concourse.mybir as mybir
    import concourse.tile as tile
    from concourse import bacc
    from concourse.bass import ds

    fp32 = mybir.dt.float32
    bf16 = mybir.dt.bfloat16
    f8e4 = mybir.dt.float8e4
    f8e5 = mybir.dt.float8e5
    u8 = mybir.dt.uint8

    nc = bacc.Bacc(None, target_bir_lowering=False, debug=False)

    xb_d = nc.declare_dram_parameter("xb", [C, N], bf16, isOutput=False)
    # xpbT = x(c-major, query half) + gamma*bv, SBUF layout [p, cb*NH + i]
    xpb_d = nc.declare_dram_parameter("xpb", [P, NCB * NH], fp32, isOutput=False)
    wq_d = nc.declare_dram_parameter("wq_pre", [P, NCB * P], bf16, isOutput=False)
    wk_d = nc.declare_dram_parameter("wk_pre", [P, NCB * 4 * P], bf16, isOutput=False)
    wv_d = nc.declare_dram_parameter("wv_pre", [P, NCB * C], bf16, isOutput=False)
    bq_d = nc.declare_dram_parameter("bq_rep", [P, 1], fp32, isOutput=False)
    bk_d = nc.declare_dram_parameter("bk_pack", [P, 1], fp32, isOutput=False)
    gm_d = nc.declare_dram_parameter("gamma_bc", [P, 1], fp32, isOutput=False)
    y_d = nc.declare_dram_parameter("y", [C, NH], bf16, isOutput=True)

    with tile.TileContext(nc) as tc:
        with (
            tc.tile_pool(name="singles", bufs=1) as singles,
            tc.tile_pool(name="epool", bufs=36) as epool,
            tc.tile_pool(name="stpool", bufs=4) as stpool,
            tc.tile_pool(name="ivpool", bufs=3) as ivpool,
            tc.tile_pool(name="pp_mm", bufs=2, space="PSUM") as pp_mm,
            tc.tile_pool(name="pp_out", bufs=4, space="PSUM") as pp_out,
        ):
            # ---- persistent SBUF tensors ----
            xb_sb = singles.tile([P, NCB, N], bf16)
            xpb_sb = singles.tile([P, NCB, NH], fp32)   # x + gamma*bv, [c,i]
            wq_sb = singles.tile([P, NCB, P], bf16)
            wk_sb = singles.tile([P, NCB, 4, P], bf16)
            wv_sb = singles.tile([P, NCB, C], bf16)
            bq_sb = singles.tile([P, 1], fp32)
            bk_sb = singles.tile([P, 1], fp32)
            gm_sb = singles.tile([P, 1], fp32)
            ebias_sb = singles.tile([P, 1], fp32)       # -EXP_K for ACT exp
            kf_sb = singles.tile([P, NQ, P], bf16)   # packed: row 32r+d, quad q
            q_sb = singles.tile([P, NH], bf16)       # q replicated in 4 groups
            # vT fp8: [p, jd, o, c] = v[c, jd*256 + o*128 + p]
            vT_sb = singles.tile([P, JD, 2, C], f8e4)
            ones_sb = singles.tile([P, 2, P], f8e4)  # all-ones stationary

            # ---- input DMAs + boot warmup, carefully ordered (see v2) ----
            warm_sb = singles.tile([P, ST], bf16)
            warm_e = singles.tile([1, 1], fp32)
            xpb_flat = xpb_sb.rearrange("p o c -> p (o c)")

            def x_dma(eng, cb, rh, csl):
                rsl = slice(rh * 64, (rh + 1) * 64)
                eng.dma_start(
                    out=xb_sb[rsl, cb, csl],
                    in_=xb_d[cb * P + rh * 64:cb * P + (rh + 1) * 64, csl],
                )

            nc.sync.dma_start(
                out=wk_sb[0:64],
                in_=wk_d[0:64].rearrange("p (o r m) -> p o r m", o=NCB, r=4),
            )
            nc.scalar.dma_start(
                out=wk_sb[64:128],
                in_=wk_d[64:128].rearrange("p (o r m) -> p o r m", o=NCB, r=4),
            )
            # first 1024 columns (both channel blocks) — kproj q0/q1 + qproj
            # t0/t1 — go out before anything else
            for c0 in (0, 512):
                for rh in range(2):
                    x_dma(nc.sync, 0, rh, ds(c0, 512))
                    x_dma(nc.scalar, 1, rh, ds(c0, 512))
            nc.scalar.dma_start(out=bk_sb[:], in_=bk_d[:])
            nc.sync.dma_start(
                out=wq_sb[:], in_=wq_d.rearrange("p (o m) -> p o m", o=NCB)
            )

            # scalar stream: dummy exp next => ACT table loads early. DVE
            # memsets come first in the vector stream. xpb (needed only by
            # the first epilogue ~35us in) goes on the gpsimd SWDGE stream.
            nc.vector.memset(warm_sb[:], 0.0)
            nc.vector.memset(ones_sb[:], 1.0)
            nc.vector.memset(ebias_sb[:], -EXP_K)
            nc.scalar.activation(
                warm_e, warm_sb[0:1, 0:1], mybir.ActivationFunctionType.Exp
            )
            # dummy matmuls keep the PE HAM activity monitor fed so real
            # matmuls run at 2.4 GHz; they overlap the input DMA window.
            for w in range(14):
                wp = pp_out.tile([P, ST], fp32, tag="out", name=f"warm_{w}")
                nc.tensor.matmul(
                    wp, warm_sb[:, 0:P], warm_sb, start=True, stop=True
                )

            for rh in range(2):
                for cc in range(4):
                    rsl = slice(rh * 64, (rh + 1) * 64)
                    csl = ds(cc * 1024, 1024)
                    nc.gpsimd.dma_start(out=xpb_flat[rsl, csl], in_=xpb_d[rsl, csl])

            # remaining x columns as 1024-col chunks, biases, wv
            nc.scalar.dma_start(out=bq_sb[:], in_=bq_d[:])
            nc.scalar.dma_start(out=gm_sb[:], in_=gm_d[:])
            for c0 in (1024, 2048, 3072):
                for rh in range(2):
                    x_dma(nc.sync, 0, rh, ds(c0, 1024))
                    x_dma(nc.scalar, 1, rh, ds(c0, 1024))
                if c0 == 1024:
                    for rh in range(2):
                        rsl = slice(rh * 64, (rh + 1) * 64)
                        nc.sync.dma_start(
                            out=wv_sb[rsl],
                            in_=wv_d[rsl].rearrange("p (o m) -> p o m", o=NCB),
                        )

            # ---- projection helpers ----
            def k_proj(q):
                kp = pp_out.tile([P, P], fp32, tag="out", name=f"kp_{q}")
                pairs = [(r, cb) for r in range(4) for cb in range(NCB)]
                for i, (r, cb) in enumerate(pairs):
                    nc.tensor.matmul(
                        kp, wk_sb[:, cb, r],
                        xb_sb[:, cb, ds((4 * q + r) * P, P)],
                        start=(i == 0), stop=(i == len(pairs) - 1),
                    )
                # bias-add on ACT (Identity w/ bias AP) to keep DVE load down
                nc.scalar.activation(
                    kf_sb[:, q, :], kp,
                    mybir.ActivationFunctionType.Identity, bias=bk_sb[:],
                )

            def v_proj_pair(t):
                # two key-blocks per PSUM tile (one bank); the fp8 cast lands
                # directly in the DoubleRow [Ki, Ko, c] interleave layout
                vp = pp_out.tile([P, 2, C], fp32, tag="out", name=f"vp_{t}")
                for u in range(2):
                    j = 2 * t + u
                    nc.tensor.matmul(
                        vp[:, u], xb_sb[:, 0, ds(j * P, P)], wv_sb[:, 0],
                        start=True, stop=False,
                    )
                    nc.tensor.matmul(
                        vp[:, u], xb_sb[:, 1, ds(j * P, P)], wv_sb[:, 1],
                        start=False, stop=True,
                    )
                nc.vector.tensor_copy(vT_sb[:, t], vp)

            def q_proj(t):
                qp = pp_out.tile([P, ST], fp32, tag="out", name=f"qp_{t}")
                nc.tensor.matmul(
                    qp, wq_sb[:, 0], xb_sb[:, 0, ds(t * ST, ST)],
                    start=True, stop=False,
                )
                nc.tensor.matmul(
                    qp, wq_sb[:, 1], xb_sb[:, 1, ds(t * ST, ST)],
                    start=False, stop=True,
                )
                nc.vector.tensor_scalar_add(q_sb[:, ds(t * ST, ST)], qp, bq_sb)

            # Minimal prefix: st0's first score quad needs only kf q0 + q t0.
            k_proj(0)
            q_proj(0)

            es_by_st = [[] for _ in range(NST)]
            vp_cnt = [0]

            def scores_quad(st_i, q):
                tiles = []
                for half in range(2):
                    sp = pp_mm.tile(
                        [P, 2, ST], fp32, tag="mm", name=f"sp_{st_i}_{q}_{half}"
                    )
                    for rr in range(2):
                        r = 2 * half + rr
                        nc.tensor.matmul(
                            sp[:, rr],
                            kf_sb[32 * r:32 * (r + 1), q, :],
                            q_sb[32 * r:32 * (r + 1), ds(st_i * ST, ST)],
                            start=True, stop=True,
                            tile_position=(32 * r, 0),
                        )
                    tiles.append(sp)
                for half, sp in enumerate(tiles):
                    # e tile [Ki=128, Ko=2, i]: j = (2q+half)*256 + Ko*128 + Ki
                    e = epool.tile(
                        [P, 2, ST], f8e5, name=f"e_{st_i}_{q}_{half}", tag="e"
                    )
                    if _use_dve(st_i, q, half):
                        # DVE: u8 = s*4*log2e + B == e5m2 bits of ~exp(s-16);
                        # saturating uint8 convert zeroes underflows.
                        nc.vector.tensor_scalar(
                            e.bitcast(u8), sp, EXP_A8, EXP_B8,
                            op0=mybir.AluOpType.mult, op1=mybir.AluOpType.add,
                        )
                    else:
                        nc.scalar.activation(
                            e, sp, mybir.ActivationFunctionType.Exp,
                            bias=ebias_sb[:],
                        )
                    es_by_st[st_i].append(e)

            # ---- st0 score/exp phase: fill the PE with the remaining K/Q
            # projections (just-in-time) and the V projection. A couple of
            # extra warm matmuls per early quad bridge the x-DMA wait so the
            # HAM clock gate never re-throttles (cold PE = half clock).
            for q in range(NQ):
                if 1 <= q <= 5:
                    # dependency-free bridge matmuls ahead of the x-waiting
                    # kproj keep the HAM activity window fed
                    for w in range(2):
                        wp = pp_out.tile(
                            [P, ST], fp32, tag="out", name=f"warmb_{q}_{w}"
                        )
                        nc.tensor.matmul(
                            wp, warm_sb[:, 0:P], warm_sb, start=True, stop=True
                        )
                if q + 1 < NQ:
                    k_proj(q + 1)
                if q in (2, 4, 6):
                    q_proj({2: 1, 4: 2, 6: 3}[q])
                scores_quad(0, q)
                while (vp_cnt[0] + 1) * 2 <= 4 * (q + 1) and vp_cnt[0] < JD:
                    v_proj_pair(vp_cnt[0])
                    vp_cnt[0] += 1

            # ---- PV phases (fp8 DoubleRow); scores/exp of the NEXT
            # supertile are woven in. Three accumulation chains per st:
            # out_c0, out_c1 (128 channels each), den (ones stationary).
            for st_i in range(NST):
                es = es_by_st[st_i]
                nxt = 0
                cnt = 0
                # den-chain FIRST so the epilogue (recip -> TT -> sTT) can
                # pipeline inside the phase instead of stalling the next st.
                den_ps = pp_out.tile([P, ST], fp32, tag="out", name=f"den_{st_i}")
                chains = [den_ps]
                for cb in range(NCB):
                    chains.append(pp_out.tile(
                        [P, ST], fp32, tag="out", name=f"out_{st_i}_{cb}"
                    ))
                # st0's e-tiles are produced JUST ahead of the PV (e-gen
                # paced), so interleave the three chains per jd there to
                # keep PE duty high; later sts have all e-tiles prebuilt and
                # run chain-serial so the epilogue pipelines within the
                # phase and PSUM banks rotate stall-free.
                if st_i == 0:
                    sched = [(ci, jd) for jd in range(JD) for ci in range(3)]
                else:
                    sched = [(ci, jd) for ci in range(3) for jd in range(JD)]
                for ci, jd in sched:
                    if ci == 0:
                        lhsT = ones_sb[:]
                    else:
                        lhsT = vT_sb[:, jd, :, ds((ci - 1) * P, P)]
                    nc.tensor.matmul(
                        chains[ci], lhsT, es[jd],
                        start=(jd == 0), stop=(jd == JD - 1),
                        perf_mode=mybir.MatmulPerfMode.DoubleRow,
                    )
                    cnt += 1
                    if cnt % 6 == 0 and st_i + 1 < NST and nxt < NQ:
                        scores_quad(st_i + 1, nxt)
                        nxt += 1
                # epilogue: y[c,i] = (out*inv)*gamma + xpbT.  inv = 1/den via
                # the fp32 bit trick (one DVE op, ~+-5%; exact at gamma=0 and
                # cheap enough that DVE never paces the PE). den is
                # replicated across partitions by the ones stationary.
                inv = ivpool.tile([P, ST], fp32)
                nc.vector.tensor_scalar(
                    inv.bitcast(mybir.dt.uint32), den_ps.bitcast(mybir.dt.uint32),
                    -1.0, RECIP_MAGIC,
                    op0=mybir.AluOpType.mult, op1=mybir.AluOpType.add,
                )
                for cb in range(NCB):
                    t1 = ivpool.tile([P, ST], fp32)
                    nc.vector.tensor_mul(t1, chains[1 + cb], inv)
                    stg = stpool.tile([P, ST], bf16)
                    nc.vector.scalar_tensor_tensor(
                        stg, t1, gm_sb, xpb_sb[:, cb, ds(st_i * ST, ST)],
                        op0=mybir.AluOpType.mult,
                        op1=mybir.AluOpType.add,
                    )
                    for rq in range(2):
                        nc.sync.dma_start(
                            out=y_d[ds(cb * P + rq * 64, 64), ds(st_i * ST, ST)],
                            in_=stg[rq * 64:(rq + 1) * 64, :],
                        )
                while st_i + 1 < NST and nxt < NQ:
                    scores_quad(st_i + 1, nxt)
                    nxt += 1

    return nc


def _get_program():
    global _PROG
    if _PROG is None:
        _PROG = _build_program()
        if not _PROG.is_finalized():
            _PROG.finalize()
    return _PROG


def kernel(x, Wq, bq, Wk, bk, Wv, bv, gamma):
    global LAST_RESULT
    import ml_dtypes
    from concourse.bass_utils import run_bass_kernel_spmd

    bf16 = ml_dtypes.bfloat16
    x = np.ascontiguousarray(np.asarray(x, dtype=np.float32))
    Wq = np.asarray(Wq, dtype=np.float32)
    bq = np.asarray(bq, dtype=np.float32)
    Wk = np.asarray(Wk, dtype=np.float32)
    bk = np.asarray(bk, dtype=np.float32)
    Wv = np.asarray(Wv, dtype=np.float32)
    bv = np.asarray(bv, dtype=np.float32)
    gamma = np.asarray(gamma, dtype=np.float32)

    # wq replicated into all four 32-row groups of the PE array
    wq_rep = np.zeros((C, P), dtype=np.float32)
    for r in range(4):
        wq_rep[:, 32 * r:32 * (r + 1)] = Wq.T
    # wk variant r carries WkT at column offset 32r (r = 0..3)
    wk_pack = np.zeros((C, 4, P), dtype=np.float32)
    for r in range(4):
        wk_pack[:, r, 32 * r:32 * (r + 1)] = Wk.T
    bq_rep = np.tile(bq, 4)[:, None].astype(np.float32)
    bk_pack = np.tile(bk, 4)[:, None].astype(np.float32)
    gval = float(gamma.reshape(-1)[0])
    gm_bc = np.full((P, 1), gval, dtype=np.float32)

    def _swz(a):
        # [C, F] -> [128, NCB*F]: exact SBUF layout (partition-major)
        f = a.reshape(NCB, P, -1)
        return np.ascontiguousarray(
            f.transpose(1, 0, 2).reshape(P, -1).astype(bf16)
        )

    wq_pre = _swz(wq_rep)
    wk_pre = _swz(wk_pack.reshape(C, 4 * P))
    wv_pre = _swz(Wv.T)

    xf = x.reshape(B, C, N)
    in_maps = []
    for core in range(8):
        b, h = core // 2, core % 2
        xb = xf[b]
        if h == 0:
            x_roll = xb
        else:
            x_roll = np.concatenate([xb[:, NH:], xb[:, :NH]], axis=1)
        # xpbT[p, cb, i] = x_roll[cb*128 + p, i] + gamma*bv[cb*128 + p]
        xq = x_roll[:, :NH] + gval * bv[:, None]
        xpb = np.ascontiguousarray(
            xq.reshape(NCB, P, NH).transpose(1, 0, 2).reshape(P, NCB * NH)
        ).astype(np.float32)
        in_maps.append({
            "xb": np.ascontiguousarray(x_roll.astype(bf16)),
            "xpb": xpb,
            "wq_pre": wq_pre,
            "wk_pre": wk_pre,
            "wv_pre": wv_pre,
            "bq_rep": bq_rep,
            "bk_pack": bk_pack,
            "gamma_bc": gm_bc,
        })

    nc = _get_program()
    res = run_bass_kernel_spmd(
        nc, in_maps, core_ids=list(range(8)),
        trace=bool(os.environ.get("BASS_TRACE")),
    )
    LAST_RESULT = res

    out = np.empty((B, C, N), dtype=np.float32)
    for core in range(8):
        b, h = core // 2, core % 2
        y = res.results[core]["y"]
        out[b][:, h * NH:(h + 1) * NH] = y.astype(np.float32)
    return out.reshape(B, C, H, W)


# revision 19
# speedup vs baseline: 1.0078x; 1.0078x over previous
"""Position-attention kernel for Trainium2 (8 NeuronCores, SPMD).

Reference computation (per batch b):
    q = Wq @ x + bq        [32, 4096]
    k = Wk @ x + bk        [32, 4096]
    v = Wv @ x + bv        [256, 4096]
    attn = softmax_j(q_i . k_j)           [4096, 4096]
    out[c, i] = sum_j v[c, j] attn[i, j]
    y = gamma * out + x

Sharding: B=4 batches x 2 query-halves -> 8 cores. Each core computes the
full softmax rows for its 2048 queries against all 4096 keys of its batch.
Host rotates x columns per core so the core's query half is always columns
0:2048 (softmax and the PV contraction are invariant to key/value column
order, as long as K and V use the same order).

Device-side structure (per core):
  - projections in bf16 (x pre-cast on host; weights pre-packed on host).
  - scores computed transposed (sT[j, i]) in PSUM, 4 key-blocks at a time
    packed into PE row-groups 0/32/64/96 via tile_position (the K=32
    contractions run concurrently in the array); kf is stored packed
    ([d + 32*r] rows), q replicated into all four row groups. Each quad's
    scores land in two 2-bank PSUM tiles (rows 0/32 and rows 64/96).
  - score->e conversion produces SHIFTED weights exp(s - 16) in fp8e5m2,
    SPLIT between the ACT engine (true exp with bias, PSUM -> fp8) and the
    DVE (log-domain affine bit trick: u8 = s*4*log2e + const = e5m2 bits of
    ~exp(s-16), saturating at 0 for underflow). The global 2^-16-ish scale
    cancels in the softmax ratio. Neither engine paces the PE.
  - PV in fp8 with DoubleRow (2 fp8 weights/PE cell, K=256 per matmul):
    out[c, i] = sum_j vT[j, c] e[j, i] with vT (fp8e4m3, [Ki=128,Ko=2,c]
    interleave) stationary and e (fp8e5m2, [Ki,Ko,i]) moving; 16 j-double-
    blocks x 2 channel blocks per supertile. A third accumulation chain
    with an all-ones stationary gives den[i] = sum_j e[j, i] replicated on
    all 128 partitions. Chains are ordered c0-chain, c1-chain, den-chain
    so PSUM out banks recycle without stalling.
  - epilogue: y[c, i] = (out[c, i] * (1/den)[c, i]) * gamma + xpbT[c, i],
    where xpbT = x + gamma*bv (the bv term works because sum_j attn = 1).
    Output written [C, NH] in bf16; host concatenates and upcasts.
  - input DMA issue is split across the sync + scalar HWDGE sequencers
    (~600ns per dma_start, 16 queues each); dummy matmuls + a dummy exp
    warm the PE HAM clock gate and the ACT function table during the DMA
    window.
"""

import os
import numpy as np

P = 128
B = 4
C = 256
CQ = 32
H = W = 64
N = H * W            # 4096 keys per batch
NH = N // 2          # 2048 queries per core
NCB = C // P         # 2 channel blocks
ST = 512             # query supertile
NST = NH // ST       # 4
JB = N // P          # 32 key blocks
JD = JB // 2         # 16 key double-blocks (DoubleRow K=256)
NQ = JB // 4         # 8 score quads per supertile

EXP_K = 16.0                      # softmax shift: weights are exp(s - 16)
EXP_A8 = 5.770780163555856        # 4 * log2(e)
EXP_B8 = 60.0 - EXP_K * EXP_A8    # e5m2 bits offset (60 = 15*4 bias)


RECIP_MAGIC = float(0x7EF127EA)   # bit-trick reciprocal seed constant


# exp offload: half-tiles where _use_dve is True are computed on the DVE.
def _use_dve(st_i, q, half):
    if half == 0:
        return False
    if st_i == 0:
        return q in (2, 5, 7)
    return q not in (1, 3)


_PROG = None         # cached build
LAST_RESULT = None   # BassKernelResults of the last run (for test harness)


def _build_program():
    import concourse.mybir as mybir
    import concourse.tile as tile
    from concourse import bacc
    from concourse.bass import ds

    fp32 = mybir.dt.float32
    bf16 = mybir.dt.bfloat16
    f8e4 = mybir.dt.float8e4
    f8e5 = mybir.dt.float8e5
    u8 = mybir.dt.uint8

    nc = bacc.Bacc(None, target_bir_lowering=False, debug=False)

    xb_d = nc.declare_dram_parameter("xb", [C, N], bf16, isOutput=False)
    # xpbT = x(c-major, query half) + gamma*bv, SBUF layout [p, cb*NH + i]
    xpb_d = nc.declare_dram_parameter("xpb", [P, NCB * NH], fp32, isOutput=False)
    wq_d = nc.declare_dram_parameter("wq_pre", [P, NCB * P], bf16, isOutput=False)
    wk_d = nc.declare_dram_parameter("wk_pre", [P, NCB * 4 * P], bf16, isOutput=False)
    wv_d = nc.declare_dram_parameter("wv_pre", [P, NCB * C], bf16, isOutput=False)
    bq_d = nc.declare_dram_parameter("bq_rep", [P, 1], fp32, isOutput=False)
    bk_d = nc.declare_dram_parameter("bk_pack", [P, 1], fp32, isOutput=False)
    gm_d = nc.declare_dram_parameter("gamma_bc", [P, 1], fp32, isOutput=False)
    y_d = nc.declare_dram_parameter("y", [C, NH], bf16, isOutput=True)

    with tile.TileContext(nc) as tc:
        with (
            tc.tile_pool(name="singles", bufs=1) as singles,
            tc.tile_pool(name="epool", bufs=36) as epool,
            tc.tile_pool(name="stpool", bufs=4) as stpool,
            tc.tile_pool(name="ivpool", bufs=3) as ivpool,
            tc.tile_pool(name="pp_mm", bufs=2, space="PSUM") as pp_mm,
            tc.tile_pool(name="pp_out", bufs=4, space="PSUM") as pp_out,
        ):
            # ---- persistent SBUF tensors ----
            xb_sb = singles.tile([P, NCB, N], bf16)
            xpb_sb = singles.tile([P, NCB, NH], fp32)   # x + gamma*bv, [c,i]
            wq_sb = singles.tile([P, NCB, P], bf16)
            wk_sb = singles.tile([P, NCB, 4, P], bf16)
            wv_sb = singles.tile([P, NCB, C], bf16)
            bq_sb = singles.tile([P, 1], fp32)
            bk_sb = singles.tile([P, 1], fp32)
            gm_sb = singles.tile([P, 1], fp32)
            ebias_sb = singles.tile([P, 1], fp32)       # -EXP_K for ACT exp
            kf_sb = singles.tile([P, NQ, P], bf16)   # packed: row 32r+d, quad q
            q_sb = singles.tile([P, NH], bf16)       # q replicated in 4 groups
            # vT fp8: [p, jd, o, c] = v[c, jd*256 + o*128 + p]
            vT_sb = singles.tile([P, JD, 2, C], f8e4)
            ones_sb = singles.tile([P, 2, P], f8e4)  # all-ones stationary

            # ---- input DMAs + boot warmup, carefully ordered (see v2) ----
            warm_sb = singles.tile([P, ST], bf16)
            warm_e = singles.tile([1, 1], fp32)
            xpb_flat = xpb_sb.rearrange("p o c -> p (o c)")

            def x_dma(eng, cb, rh, csl):
                rsl = slice(rh * 64, (rh + 1) * 64)
                eng.dma_start(
                    out=xb_sb[rsl, cb, csl],
                    in_=xb_d[cb * P + rh * 64:cb * P + (rh + 1) * 64, csl],
                )

            nc.sync.dma_start(
                out=wk_sb[0:64],
                in_=wk_d[0:64].rearrange("p (o r m) -> p o r m", o=NCB, r=4),
            )
            nc.scalar.dma_start(
                out=wk_sb[64:128],
                in_=wk_d[64:128].rearrange("p (o r m) -> p o r m", o=NCB, r=4),
            )
            # first 512 columns (both channel blocks) — kproj q0 + qproj t0
            for rh in range(2):
                x_dma(nc.sync, 0, rh, ds(0, 512))
                x_dma(nc.scalar, 1, rh, ds(0, 512))

            # scalar stream: dummy exp next => ACT table loads early (the
            # first real exp gates all of st0's e-gen). DVE memsets come
            # first in the vector stream. xpb (needed only by the first
            # epilogue ~35us in) goes on the gpsimd SWDGE stream.
            nc.vector.memset(warm_sb[:], 0.0)
            nc.vector.memset(ones_sb[:], 1.0)
            nc.vector.memset(ebias_sb[:], -EXP_K)
            nc.scalar.activation(
                warm_e, warm_sb[0:1, 0:1], mybir.ActivationFunctionType.Exp
            )
            nc.scalar.dma_start(out=bk_sb[:], in_=bk_d[:])
            nc.sync.dma_start(
                out=wq_sb[:], in_=wq_d.rearrange("p (o m) -> p o m", o=NCB)
            )
            # dummy matmuls keep the PE HAM activity monitor fed so real
            # matmuls run at 2.4 GHz; they overlap the input DMA window.
            for w in range(14):
                wp = pp_out.tile([P, ST], fp32, tag="out", name=f"warm_{w}")
                nc.tensor.matmul(
                    wp, warm_sb[:, 0:P], warm_sb, start=True, stop=True
                )

            for rh in range(2):
                for cc in range(4):
                    rsl = slice(rh * 64, (rh + 1) * 64)
                    csl = ds(cc * 1024, 1024)
                    nc.gpsimd.dma_start(out=xpb_flat[rsl, csl], in_=xpb_d[rsl, csl])

            # remaining x columns, biases, wv
            nc.scalar.dma_start(out=bq_sb[:], in_=bq_d[:])
            for c0, cw in ((512, 512), (1024, 1024), (2048, 1024), (3072, 1024)):
                for rh in range(2):
                    x_dma(nc.sync, 0, rh, ds(c0, cw))
                    x_dma(nc.scalar, 1, rh, ds(c0, cw))
                if c0 == 1024:
                    for rh in range(2):
                        rsl = slice(rh * 64, (rh + 1) * 64)
                        nc.sync.dma_start(
                            out=wv_sb[rsl],
                            in_=wv_d[rsl].rearrange("p (o m) -> p o m", o=NCB),
                        )
            nc.scalar.dma_start(out=gm_sb[:], in_=gm_d[:])

            # ---- projection helpers ----
            def k_proj(q):
                kp = pp_out.tile([P, P], fp32, tag="out", name=f"kp_{q}")
                pairs = [(r, cb) for r in range(4) for cb in range(NCB)]
                for i, (r, cb) in enumerate(pairs):
                    nc.tensor.matmul(
                        kp, wk_sb[:, cb, r],
                        xb_sb[:, cb, ds((4 * q + r) * P, P)],
                        start=(i == 0), stop=(i == len(pairs) - 1),
                    )
                # bias-add on ACT (Identity w/ bias AP) to keep DVE load down
                nc.scalar.activation(
                    kf_sb[:, q, :], kp,
                    mybir.ActivationFunctionType.Identity, bias=bk_sb[:],
                )

            def v_proj_pair(t):
                # two key-blocks per PSUM tile (one bank); the fp8 cast lands
                # directly in the DoubleRow [Ki, Ko, c] interleave layout
                vp = pp_out.tile([P, 2, C], fp32, tag="out", name=f"vp_{t}")
                for u in range(2):
                    j = 2 * t + u
                    nc.tensor.matmul(
                        vp[:, u], xb_sb[:, 0, ds(j * P, P)], wv_sb[:, 0],
                        start=True, stop=False,
                    )
                    nc.tensor.matmul(
                        vp[:, u], xb_sb[:, 1, ds(j * P, P)], wv_sb[:, 1],
                        start=False, stop=True,
                    )
                nc.vector.tensor_copy(vT_sb[:, t], vp)

            def q_proj(t):
                qp = pp_out.tile([P, ST], fp32, tag="out", name=f"qp_{t}")
                nc.tensor.matmul(
                    qp, wq_sb[:, 0], xb_sb[:, 0, ds(t * ST, ST)],
                    start=True, stop=False,
                )
                nc.tensor.matmul(
                    qp, wq_sb[:, 1], xb_sb[:, 1, ds(t * ST, ST)],
                    start=False, stop=True,
                )
                nc.vector.tensor_scalar_add(q_sb[:, ds(t * ST, ST)], qp, bq_sb)

            # Minimal prefix: st0's first score quad needs only kf q0 + q t0.
            k_proj(0)
            q_proj(0)

            es_by_st = [[] for _ in range(NST)]
            vp_cnt = [0]

            def scores_quad(st_i, q):
                tiles = []
                for half in range(2):
                    sp = pp_mm.tile(
                        [P, 2, ST], fp32, tag="mm", name=f"sp_{st_i}_{q}_{half}"
                    )
                    for rr in range(2):
                        r = 2 * half + rr
                        nc.tensor.matmul(
                            sp[:, rr],
                            kf_sb[32 * r:32 * (r + 1), q, :],
                            q_sb[32 * r:32 * (r + 1), ds(st_i * ST, ST)],
                            start=True, stop=True,
                            tile_position=(32 * r, 0),
                        )
                    tiles.append(sp)
                for half, sp in enumerate(tiles):
                    # e tile [Ki=128, Ko=2, i]: j = (2q+half)*256 + Ko*128 + Ki
                    e = epool.tile(
                        [P, 2, ST], f8e5, name=f"e_{st_i}_{q}_{half}", tag="e"
                    )
                    if _use_dve(st_i, q, half):
                        # DVE: u8 = s*4*log2e + B == e5m2 bits of ~exp(s-16);
                        # saturating uint8 convert zeroes underflows.
                        nc.vector.tensor_scalar(
                            e.bitcast(u8), sp, EXP_A8, EXP_B8,
                            op0=mybir.AluOpType.mult, op1=mybir.AluOpType.add,
                        )
                    else:
                        nc.scalar.activation(
                            e, sp, mybir.ActivationFunctionType.Exp,
                            bias=ebias_sb[:],
                        )
                    es_by_st[st_i].append(e)

            # ---- st0 score/exp phase: fill the PE with the remaining K/Q
            # projections (just-in-time) and the V projection. A couple of
            # extra warm matmuls per early quad bridge the x-DMA wait so the
            # HAM clock gate never re-throttles (cold PE = half clock).
            for q in range(NQ):
                if 1 <= q <= 5:
                    # dependency-free bridge matmuls ahead of the x-waiting
                    # kproj keep the HAM activity window fed
                    for w in range(2):
                        wp = pp_out.tile(
                            [P, ST], fp32, tag="out", name=f"warmb_{q}_{w}"
                        )
                        nc.tensor.matmul(
                            wp, warm_sb[:, 0:P], warm_sb, start=True, stop=True
                        )
                if q + 1 < NQ:
                    k_proj(q + 1)
                if q in (2, 4, 6):
                    q_proj({2: 1, 4: 2, 6: 3}[q])
                scores_quad(0, q)
                while (vp_cnt[0] + 1) * 2 <= 4 * (q + 1) and vp_cnt[0] < JD:
                    v_proj_pair(vp_cnt[0])
                    vp_cnt[0] += 1

            # ---- PV phases (fp8 DoubleRow); scores/exp of the NEXT
            # supertile are woven in. Three accumulation chains per st:
            # out_c0, out_c1 (128 channels each), den (ones stationary).
            for st_i in range(NST):
                es = es_by_st[st_i]
                nxt = 0
                cnt = 0
                # den-chain FIRST so the epilogue (recip -> TT -> sTT) can
                # pipeline inside the phase instead of stalling the next st.
                den_ps = pp_out.tile([P, ST], fp32, tag="out", name=f"den_{st_i}")
                chains = [den_ps]
                for cb in range(NCB):
                    chains.append(pp_out.tile(
                        [P, ST], fp32, tag="out", name=f"out_{st_i}_{cb}"
                    ))
                # st0's e-tiles are produced JUST ahead of the PV (e-gen
                # paced), so interleave the three chains per jd there to
                # keep PE duty high; later sts have all e-tiles prebuilt and
                # run chain-serial so the epilogue pipelines within the
                # phase and PSUM banks rotate stall-free.
                if st_i == 0:
                    sched = [(ci, jd) for jd in range(JD) for ci in range(3)]
                else:
                    sched = [(ci, jd) for ci in range(3) for jd in range(JD)]
                for ci, jd in sched:
                    if ci == 0:
                        lhsT = ones_sb[:]
                    else:
                        lhsT = vT_sb[:, jd, :, ds((ci - 1) * P, P)]
                    nc.tensor.matmul(
                        chains[ci], lhsT, es[jd],
                        start=(jd == 0), stop=(jd == JD - 1),
                        perf_mode=mybir.MatmulPerfMode.DoubleRow,
                    )
                    cnt += 1
                    if cnt % 6 == 0 and st_i + 1 < NST and nxt < NQ:
                        scores_quad(st_i + 1, nxt)
                        nxt += 1
                # epilogue: y[c,i] = (out*inv)*gamma + xpbT.  inv = 1/den via
                # the fp32 bit trick (one DVE op, ~+-5%; exact at gamma=0 and
                # cheap enough that DVE never paces the PE). den is
                # replicated across partitions by the ones stationary.
                inv = ivpool.tile([P, ST], fp32)
                nc.vector.tensor_scalar(
                    inv.bitcast(mybir.dt.uint32), den_ps.bitcast(mybir.dt.uint32),
                    -1.0, RECIP_MAGIC,
                    op0=mybir.AluOpType.mult, op1=mybir.AluOpType.add,
                )
                for cb in range(NCB):
                    t1 = ivpool.tile([P, ST], fp32)
                    nc.vector.tensor_mul(t1, chains[1 + cb], inv)
                    stg = stpool.tile([P, ST], bf16)
                    nc.vector.scalar_tensor_tensor(
                        stg, t1, gm_sb, xpb_sb[:, cb, ds(st_i * ST, ST)],
                        op0=mybir.AluOpType.mult,
                        op1=mybir.AluOpType.add,
                    )
                    for rq in range(2):
                        nc.sync.dma_start(
                            out=y_d[ds(cb * P + rq * 64, 64), ds(st_i * ST, ST)],
                            in_=stg[rq * 64:(rq + 1) * 64, :],
                        )
                while st_i + 1 < NST and nxt < NQ:
                    scores_quad(st_i + 1, nxt)
                    nxt += 1

    return nc


def _get_program():
    global _PROG
    if _PROG is None:
        _PROG = _build_program()
        if not _PROG.is_finalized():
            _PROG.finalize()
    return _PROG


def kernel(x, Wq, bq, Wk, bk, Wv, bv, gamma):
    global LAST_RESULT
    import ml_dtypes
    from concourse.bass_utils import run_bass_kernel_spmd

    bf16 = ml_dtypes.bfloat16
    x = np.ascontiguousarray(np.asarray(x, dtype=np.float32))
    Wq = np.asarray(Wq, dtype=np.float32)
    bq = np.asarray(bq, dtype=np.float32)
    Wk = np.asarray(Wk, dtype=np.float32)
    bk = np.asarray(bk, dtype=np.float32)
    Wv = np.asarray(Wv, dtype=np.float32)
    bv = np.asarray(bv, dtype=np.float32)
    gamma = np.asarray(gamma, dtype=np.float32)

    # wq replicated into all four 32-row groups of the PE array
    wq_rep = np.zeros((C, P), dtype=np.float32)
    for r in range(4):
        wq_rep[:, 32 * r:32 * (r + 1)] = Wq.T
    # wk variant r carries WkT at column offset 32r (r = 0..3)
    wk_pack = np.zeros((C, 4, P), dtype=np.float32)
    for r in range(4):
        wk_pack[:, r, 32 * r:32 * (r + 1)] = Wk.T
    bq_rep = np.tile(bq, 4)[:, None].astype(np.float32)
    bk_pack = np.tile(bk, 4)[:, None].astype(np.float32)
    gval = float(gamma.reshape(-1)[0])
    gm_bc = np.full((P, 1), gval, dtype=np.float32)

    def _swz(a):
        # [C, F] -> [128, NCB*F]: exact SBUF layout (partition-major)
        f = a.reshape(NCB, P, -1)
        return np.ascontiguousarray(
            f.transpose(1, 0, 2).reshape(P, -1).astype(bf16)
        )

    wq_pre = _swz(wq_rep)
    wk_pre = _swz(wk_pack.reshape(C, 4 * P))
    wv_pre = _swz(Wv.T)

    xf = x.reshape(B, C, N)
    in_maps = []
    for core in range(8):
        b, h = core // 2, core % 2
        xb = xf[b]
        if h == 0:
            x_roll = xb
        else:
            x_roll = np.concatenate([xb[:, NH:], xb[:, :NH]], axis=1)
        # xpbT[p, cb, i] = x_roll[cb*128 + p, i] + gamma*bv[cb*128 + p]
        xq = x_roll[:, :NH] + gval * bv[:, None]
        xpb = np.ascontiguousarray(
            xq.reshape(NCB, P, NH).transpose(1, 0, 2).reshape(P, NCB * NH)
        ).astype(np.float32)
        in_maps.append({
            "xb": np.ascontiguousarray(x_roll.astype(bf16)),
            "xpb": xpb,
            "wq_pre": wq_pre,
            "wk_pre": wk_pre,
            "wv_pre": wv_pre,
            "bq_rep": bq_rep,
            "bk_pack": bk_pack,
            "gamma_bc": gm_bc,
        })

    nc = _get_program()
    res = run_bass_kernel_spmd(
        nc, in_maps, core_ids=list(range(8)),
        trace=bool(os.environ.get("BASS_TRACE")),
    )
    LAST_RESULT = res

    out = np.empty((B, C, N), dtype=np.float32)
    for core in range(8):
        b, h = core // 2, core % 2
        y = res.results[core]["y"]
        out[b][:, h * NH:(h + 1) * NH] = y.astype(np.float32)
    return out.reshape(B, C, H, W)


# revision 21
# speedup vs baseline: 1.0556x; 1.0474x over previous
"""Position-attention kernel for Trainium2 (8 NeuronCores, SPMD).

Reference computation (per batch b):
    q = Wq @ x + bq        [32, 4096]
    k = Wk @ x + bk        [32, 4096]
    v = Wv @ x + bv        [256, 4096]
    attn = softmax_j(q_i . k_j)           [4096, 4096]
    out[c, i] = sum_j v[c, j] attn[i, j]
    y = gamma * out + x

Sharding: B=4 batches x 2 query-halves -> 8 cores. Each core computes the
full softmax rows for its 2048 queries against all 4096 keys of its batch.
Host rotates x columns per core so the core's query half is always columns
0:2048 (softmax and the PV contraction are invariant to key/value column
order, as long as K and V use the same order).

Device-side structure (per core):
  - projections in bf16 (x pre-cast on host; weights pre-packed on host).
  - scores computed transposed (sT[j, i]) in PSUM, 4 key-blocks at a time
    packed into PE row-groups 0/32/64/96 via tile_position (the K=32
    contractions run concurrently in the array); kf is stored packed
    ([d + 32*r] rows), q replicated into all four row groups. Each quad's
    scores land in two 2-bank PSUM tiles (rows 0/32 and rows 64/96).
  - score->e conversion produces SHIFTED weights exp(s - 16) in fp8e5m2,
    SPLIT between the ACT engine (true exp with bias, PSUM -> fp8) and the
    DVE (log-domain affine bit trick: u8 = s*4*log2e + const = e5m2 bits of
    ~exp(s-16), saturating at 0 for underflow). The global 2^-16-ish scale
    cancels in the softmax ratio. Neither engine paces the PE.
  - PV in fp8 with DoubleRow (2 fp8 weights/PE cell, K=256 per matmul):
    out[c, i] = sum_j vT[j, c] e[j, i] with vT (fp8e4m3, [Ki=128,Ko=2,c]
    interleave) stationary and e (fp8e5m2, [Ki,Ko,i]) moving; 16 j-double-
    blocks x 2 channel blocks per supertile. A third accumulation chain
    with an all-ones stationary gives den[i] = sum_j e[j, i] replicated on
    all 128 partitions. Chains are ordered c0-chain, c1-chain, den-chain
    so PSUM out banks recycle without stalling.
  - epilogue: y[c, i] = (out[c, i] * (1/den)[c, i]) * gamma + xpbT[c, i],
    where xpbT = x + gamma*bv (the bv term works because sum_j attn = 1).
    Output written [C, NH] in bf16; host concatenates and upcasts.
  - input DMA issue is split across the sync + scalar HWDGE sequencers
    (~600ns per dma_start, 16 queues each); dummy matmuls + a dummy exp
    warm the PE HAM clock gate and the ACT function table during the DMA
    window.
"""

import os
import numpy as np

P = 128
B = 4
C = 256
CQ = 32
H = W = 64
N = H * W            # 4096 keys per batch
NH = N // 2          # 2048 queries per core
NCB = C // P         # 2 channel blocks
ST = 512             # query supertile
NST = NH // ST       # 4
JB = N // P          # 32 key blocks
JD = JB // 2         # 16 key double-blocks (DoubleRow K=256)
NQ = JB // 4         # 8 score quads per supertile

EXP_K = 16.0                      # softmax shift: weights are exp(s - 16)
EXP_A8 = 5.770780163555856        # 4 * log2(e)
EXP_B8 = 60.0 - EXP_K * EXP_A8    # e5m2 bits offset (60 = 15*4 bias)


RECIP_MAGIC = float(0x7EF127EA)   # bit-trick reciprocal seed constant


# exp offload: half-tiles where _use_dve is True are computed on the DVE.
def _use_dve(st_i, q, half):
    if half == 0:
        return False
    if st_i == 0:
        return q in (2, 5, 7)
    if st_i == 2:
        return q not in (3, 5)
    return q != 3


_PROG = None         # cached build
LAST_RESULT = None   # BassKernelResults of the last run (for test harness)


def _build_program():
    import concourse.mybir as mybir
    import concourse.tile as tile
    from concourse import bacc
    from concourse.bass import ds

    fp32 = mybir.dt.float32
    bf16 = mybir.dt.bfloat16
    f8e4 = mybir.dt.float8e4
    f8e5 = mybir.dt.float8e5
    u8 = mybir.dt.uint8

    nc = bacc.Bacc(None, target_bir_lowering=False, debug=False)

    xb_d = nc.declare_dram_parameter("xb", [C, N], bf16, isOutput=False)
    # xpbT = x(c-major, query half) + gamma*bv, SBUF layout [p, cb*NH + i]
    xpb_d = nc.declare_dram_parameter("xpb", [P, NCB * NH], fp32, isOutput=False)
    wq_d = nc.declare_dram_parameter("wq_pre", [P, NCB * P], bf16, isOutput=False)
    wk_d = nc.declare_dram_parameter("wk_pre", [P, NCB * 4 * P], bf16, isOutput=False)
    wv_d = nc.declare_dram_parameter("wv_pre", [P, NCB * C], bf16, isOutput=False)
    bq_d = nc.declare_dram_parameter("bq_rep", [P, 1], fp32, isOutput=False)
    bk_d = nc.declare_dram_parameter("bk_pack", [P, 1], fp32, isOutput=False)
    gm_d = nc.declare_dram_parameter("gamma_bc", [P, 1], fp32, isOutput=False)
    y_d = nc.declare_dram_parameter("y", [C, NH], bf16, isOutput=True)

    with tile.TileContext(nc) as tc:
        with (
            tc.tile_pool(name="singles", bufs=1) as singles,
            tc.tile_pool(name="epool", bufs=36) as epool,
            tc.tile_pool(name="stpool", bufs=4) as stpool,
            tc.tile_pool(name="ivpool", bufs=3) as ivpool,
            tc.tile_pool(name="pp_mm", bufs=2, space="PSUM") as pp_mm,
            tc.tile_pool(name="pp_out", bufs=4, space="PSUM") as pp_out,
        ):
            # ---- persistent SBUF tensors ----
            xb_sb = singles.tile([P, NCB, N], bf16)
            xpb_sb = singles.tile([P, NCB, NH], fp32)   # x + gamma*bv, [c,i]
            wq_sb = singles.tile([P, NCB, P], bf16)
            wk_sb = singles.tile([P, NCB, 4, P], bf16)
            wv_sb = singles.tile([P, NCB, C], bf16)
            bq_sb = singles.tile([P, 1], fp32)
            bk_sb = singles.tile([P, 1], fp32)
            gm_sb = singles.tile([P, 1], fp32)
            ebias_sb = singles.tile([P, 1], fp32)       # -EXP_K for ACT exp
            kf_sb = singles.tile([P, NQ, P], bf16)   # packed: row 32r+d, quad q
            q_sb = singles.tile([P, NH], bf16)       # q replicated in 4 groups
            # vT fp8: [p, jd, o, c] = v[c, jd*256 + o*128 + p]
            vT_sb = singles.tile([P, JD, 2, C], f8e4)
            ones_sb = singles.tile([P, 2, P], f8e4)  # all-ones stationary

            # ---- input DMAs + boot warmup, carefully ordered (see v2) ----
            warm_sb = singles.tile([P, ST], bf16)
            warm_e = singles.tile([1, 1], fp32)
            xpb_flat = xpb_sb.rearrange("p o c -> p (o c)")

            def x_dma(eng, cb, rh, csl):
                rsl = slice(rh * 64, (rh + 1) * 64)
                eng.dma_start(
                    out=xb_sb[rsl, cb, csl],
                    in_=xb_d[cb * P + rh * 64:cb * P + (rh + 1) * 64, csl],
                )

            nc.sync.dma_start(
                out=wk_sb[0:64],
                in_=wk_d[0:64].rearrange("p (o r m) -> p o r m", o=NCB, r=4),
            )
            nc.scalar.dma_start(
                out=wk_sb[64:128],
                in_=wk_d[64:128].rearrange("p (o r m) -> p o r m", o=NCB, r=4),
            )
            # first 512 columns (both channel blocks) — kproj q0 + qproj t0
            for rh in range(2):
                x_dma(nc.sync, 0, rh, ds(0, 512))
                x_dma(nc.scalar, 1, rh, ds(0, 512))
            nc.sync.dma_start(
                out=wq_sb[:], in_=wq_d.rearrange("p (o m) -> p o m", o=NCB)
            )

            # scalar stream: dummy exp next => ACT table loads early. DVE
            # memsets come first in the vector stream.
            nc.vector.memset(warm_sb[:], 0.0)
            nc.vector.memset(ones_sb[:], 1.0)
            nc.vector.memset(ebias_sb[:], -EXP_K)
            nc.scalar.activation(
                warm_e, warm_sb[0:1, 0:1], mybir.ActivationFunctionType.Exp
            )
            # dummy matmuls keep the PE HAM activity monitor fed so real
            # matmuls run at 2.4 GHz; they overlap the input DMA window.
            for w in range(14):
                wp = pp_out.tile([P, ST], fp32, tag="out", name=f"warm_{w}")
                nc.tensor.matmul(
                    wp, warm_sb[:, 0:P], warm_sb, start=True, stop=True
                )

            # remaining x columns as 1024-col chunks, biases, wv, xpb
            nc.scalar.dma_start(out=bq_sb[:], in_=bq_d[:])
            nc.scalar.dma_start(out=bk_sb[:], in_=bk_d[:])
            nc.scalar.dma_start(out=gm_sb[:], in_=gm_d[:])
            for c0, cw in ((512, 512), (1024, 1024), (2048, 1024), (3072, 1024)):
                for rh in range(2):
                    x_dma(nc.sync, 0, rh, ds(c0, cw))
                    x_dma(nc.scalar, 1, rh, ds(c0, cw))
                if c0 == 1024:
                    for rh in range(2):
                        rsl = slice(rh * 64, (rh + 1) * 64)
                        nc.sync.dma_start(
                            out=wv_sb[rsl],
                            in_=wv_d[rsl].rearrange("p (o m) -> p o m", o=NCB),
                        )
            for rh in range(2):
                for cc in range(4):
                    rsl = slice(rh * 64, (rh + 1) * 64)
                    csl = ds(cc * 1024, 1024)
                    eng = nc.sync if (cc % 2 == 0) else nc.scalar
                    eng.dma_start(out=xpb_flat[rsl, csl], in_=xpb_d[rsl, csl])

            # ---- projection helpers ----
            def k_proj(q):
                kp = pp_out.tile([P, P], fp32, tag="out", name=f"kp_{q}")
                pairs = [(r, cb) for r in range(4) for cb in range(NCB)]
                for i, (r, cb) in enumerate(pairs):
                    nc.tensor.matmul(
                        kp, wk_sb[:, cb, r],
                        xb_sb[:, cb, ds((4 * q + r) * P, P)],
                        start=(i == 0), stop=(i == len(pairs) - 1),
                    )
                # bias-add on ACT (Identity w/ bias AP) to keep DVE load down
                nc.scalar.activation(
                    kf_sb[:, q, :], kp,
                    mybir.ActivationFunctionType.Identity, bias=bk_sb[:],
                )

            def v_proj_pair(t):
                # two key-blocks per PSUM tile (one bank); the fp8 cast lands
                # directly in the DoubleRow [Ki, Ko, c] interleave layout
                vp = pp_out.tile([P, 2, C], fp32, tag="out", name=f"vp_{t}")
                for u in range(2):
                    j = 2 * t + u
                    nc.tensor.matmul(
                        vp[:, u], xb_sb[:, 0, ds(j * P, P)], wv_sb[:, 0],
                        start=True, stop=False,
                    )
                    nc.tensor.matmul(
                        vp[:, u], xb_sb[:, 1, ds(j * P, P)], wv_sb[:, 1],
                        start=False, stop=True,
                    )
                nc.vector.tensor_copy(vT_sb[:, t], vp)

            def q_proj(t):
                qp = pp_out.tile([P, ST], fp32, tag="out", name=f"qp_{t}")
                nc.tensor.matmul(
                    qp, wq_sb[:, 0], xb_sb[:, 0, ds(t * ST, ST)],
                    start=True, stop=False,
                )
                nc.tensor.matmul(
                    qp, wq_sb[:, 1], xb_sb[:, 1, ds(t * ST, ST)],
                    start=False, stop=True,
                )
                nc.vector.tensor_scalar_add(q_sb[:, ds(t * ST, ST)], qp, bq_sb)

            # Minimal prefix: st0's first score quad needs only kf q0 + q t0.
            k_proj(0)
            q_proj(0)

            es_by_st = [[] for _ in range(NST)]
            vp_cnt = [0]

            def scores_quad(st_i, q):
                tiles = []
                for half in range(2):
                    sp = pp_mm.tile(
                        [P, 2, ST], fp32, tag="mm", name=f"sp_{st_i}_{q}_{half}"
                    )
                    for rr in range(2):
                        r = 2 * half + rr
                        nc.tensor.matmul(
                            sp[:, rr],
                            kf_sb[32 * r:32 * (r + 1), q, :],
                            q_sb[32 * r:32 * (r + 1), ds(st_i * ST, ST)],
                            start=True, stop=True,
                            tile_position=(32 * r, 0),
                        )
                    tiles.append(sp)
                for half, sp in enumerate(tiles):
                    # e tile [Ki=128, Ko=2, i]: j = (2q+half)*256 + Ko*128 + Ki
                    e = epool.tile(
                        [P, 2, ST], f8e5, name=f"e_{st_i}_{q}_{half}", tag="e"
                    )
                    if _use_dve(st_i, q, half):
                        # DVE: u8 = s*4*log2e + B == e5m2 bits of ~exp(s-16);
                        # saturating uint8 convert zeroes underflows.
                        nc.vector.tensor_scalar(
                            e.bitcast(u8), sp, EXP_A8, EXP_B8,
                            op0=mybir.AluOpType.mult, op1=mybir.AluOpType.add,
                        )
                    else:
                        nc.scalar.activation(
                            e, sp, mybir.ActivationFunctionType.Exp,
                            bias=ebias_sb[:],
                        )
                    es_by_st[st_i].append(e)

            # ---- st0 score/exp phase: fill the PE with the remaining K/Q
            # projections (just-in-time) and the V projection. A couple of
            # extra warm matmuls per early quad bridge the x-DMA wait so the
            # HAM clock gate never re-throttles (cold PE = half clock).
            for q in range(NQ):
                if 1 <= q <= 5:
                    # dependency-free bridge matmuls ahead of the x-waiting
                    # kproj keep the HAM activity window fed
                    for w in range(2):
                        wp = pp_out.tile(
                            [P, ST], fp32, tag="out", name=f"warmb_{q}_{w}"
                        )
                        nc.tensor.matmul(
                            wp, warm_sb[:, 0:P], warm_sb, start=True, stop=True
                        )
                if q + 1 < NQ:
                    k_proj(q + 1)
                if q in (2, 4, 6):
                    q_proj({2: 1, 4: 2, 6: 3}[q])
                scores_quad(0, q)
                while (vp_cnt[0] + 1) * 2 <= 4 * (q + 1) and vp_cnt[0] < JD:
                    v_proj_pair(vp_cnt[0])
                    vp_cnt[0] += 1

            # ---- PV phases (fp8 DoubleRow); scores/exp of the NEXT
            # supertile are woven in. Three accumulation chains per st:
            # out_c0, out_c1 (128 channels each), den (ones stationary).
            for st_i in range(NST):
                es = es_by_st[st_i]
                nxt = 0
                cnt = 0
                # den-chain FIRST so the epilogue (recip -> TT -> sTT) can
                # pipeline inside the phase instead of stalling the next st.
                den_ps = pp_out.tile([P, ST], fp32, tag="out", name=f"den_{st_i}")
                chains = [den_ps]
                for cb in range(NCB):
                    chains.append(pp_out.tile(
                        [P, ST], fp32, tag="out", name=f"out_{st_i}_{cb}"
                    ))
                # st0's e-tiles are produced JUST ahead of the PV (e-gen
                # paced), so interleave the three chains per jd there to
                # keep PE duty high; later sts have all e-tiles prebuilt and
                # run chain-serial so the epilogue pipelines within the
                # phase and PSUM banks rotate stall-free.
                if st_i == 0:
                    sched = [(ci, jd) for jd in range(JD) for ci in range(3)]
                else:
                    sched = [(ci, jd) for ci in range(3) for jd in range(JD)]
                for ci, jd in sched:
                    if ci == 0:
                        lhsT = ones_sb[:]
                    else:
                        lhsT = vT_sb[:, jd, :, ds((ci - 1) * P, P)]
                    nc.tensor.matmul(
                        chains[ci], lhsT, es[jd],
                        start=(jd == 0), stop=(jd == JD - 1),
                        perf_mode=mybir.MatmulPerfMode.DoubleRow,
                    )
                    cnt += 1
                    if cnt % 6 == 0 and st_i + 1 < NST and nxt < NQ:
                        scores_quad(st_i + 1, nxt)
                        nxt += 1
                # epilogue: y[c,i] = (out*inv)*gamma + xpbT.  inv = 1/den via
                # the fp32 bit trick (one DVE op, ~+-5%; exact at gamma=0 and
                # cheap enough that DVE never paces the PE). den is
                # replicated across partitions by the ones stationary.
                inv = ivpool.tile([P, ST], fp32)
                nc.vector.tensor_scalar(
                    inv.bitcast(mybir.dt.uint32), den_ps.bitcast(mybir.dt.uint32),
                    -1.0, RECIP_MAGIC,
                    op0=mybir.AluOpType.mult, op1=mybir.AluOpType.add,
                )
                for cb in range(NCB):
                    t1 = ivpool.tile([P, ST], fp32)
                    nc.vector.tensor_mul(t1, chains[1 + cb], inv)
                    stg = stpool.tile([P, ST], bf16)
                    nc.vector.scalar_tensor_tensor(
                        stg, t1, gm_sb, xpb_sb[:, cb, ds(st_i * ST, ST)],
                        op0=mybir.AluOpType.mult,
                        op1=mybir.AluOpType.add,
                    )
                    for rq in range(2):
                        nc.sync.dma_start(
                            out=y_d[ds(cb * P + rq * 64, 64), ds(st_i * ST, ST)],
                            in_=stg[rq * 64:(rq + 1) * 64, :],
                        )
                while st_i + 1 < NST and nxt < NQ:
                    scores_quad(st_i + 1, nxt)
                    nxt += 1

    return nc


def _get_program():
    global _PROG
    if _PROG is None:
        _PROG = _build_program()
        if not _PROG.is_finalized():
            _PROG.finalize()
    return _PROG


def kernel(x, Wq, bq, Wk, bk, Wv, bv, gamma):
    global LAST_RESULT
    import ml_dtypes
    from concourse.bass_utils import run_bass_kernel_spmd

    bf16 = ml_dtypes.bfloat16
    x = np.ascontiguousarray(np.asarray(x, dtype=np.float32))
    Wq = np.asarray(Wq, dtype=np.float32)
    bq = np.asarray(bq, dtype=np.float32)
    Wk = np.asarray(Wk, dtype=np.float32)
    bk = np.asarray(bk, dtype=np.float32)
    Wv = np.asarray(Wv, dtype=np.float32)
    bv = np.asarray(bv, dtype=np.float32)
    gamma = np.asarray(gamma, dtype=np.float32)

    # wq replicated into all four 32-row groups of the PE array
    wq_rep = np.zeros((C, P), dtype=np.float32)
    for r in range(4):
        wq_rep[:, 32 * r:32 * (r + 1)] = Wq.T
    # wk variant r carries WkT at column offset 32r (r = 0..3)
    wk_pack = np.zeros((C, 4, P), dtype=np.float32)
    for r in range(4):
        wk_pack[:, r, 32 * r:32 * (r + 1)] = Wk.T
    bq_rep = np.tile(bq, 4)[:, None].astype(np.float32)
    bk_pack = np.tile(bk, 4)[:, None].astype(np.float32)
    gval = float(gamma.reshape(-1)[0])
    gm_bc = np.full((P, 1), gval, dtype=np.float32)

    def _swz(a):
        # [C, F] -> [128, NCB*F]: exact SBUF layout (partition-major)
        f = a.reshape(NCB, P, -1)
        return np.ascontiguousarray(
            f.transpose(1, 0, 2).reshape(P, -1).astype(bf16)
        )

    wq_pre = _swz(wq_rep)
    wk_pre = _swz(wk_pack.reshape(C, 4 * P))
    wv_pre = _swz(Wv.T)

    xf = x.reshape(B, C, N)
    in_maps = []
    for core in range(8):
        b, h = core // 2, core % 2
        xb = xf[b]
        if h == 0:
            x_roll = xb
        else:
            x_roll = np.concatenate([xb[:, NH:], xb[:, :NH]], axis=1)
        # xpbT[p, cb, i] = x_roll[cb*128 + p, i] + gamma*bv[cb*128 + p]
        xq = x_roll[:, :NH] + gval * bv[:, None]
        xpb = np.ascontiguousarray(
            xq.reshape(NCB, P, NH).transpose(1, 0, 2).reshape(P, NCB * NH)
        ).astype(np.float32)
        in_maps.append({
            "xb": np.ascontiguousarray(x_roll.astype(bf16)),
            "xpb": xpb,
            "wq_pre": wq_pre,
            "wk_pre": wk_pre,
            "wv_pre": wv_pre,
            "bq_rep": bq_rep,
            "bk_pack": bk_pack,
            "gamma_bc": gm_bc,
        })

    nc = _get_program()
    res = run_bass_kernel_spmd(
        nc, in_maps, core_ids=list(range(8)),
        trace=bool(os.environ.get("BASS_TRACE")),
    )
    LAST_RESULT = res

    out = np.empty((B, C, N), dtype=np.float32)
    for core in range(8):
        b, h = core // 2, core % 2
        y = res.results[core]["y"]
        out[b][:, h * NH:(h + 1) * NH] = y.astype(np.float32)
    return out.reshape(B, C, H, W)
